# revision 1
# baseline (speedup 1.0000x reference)
"""Trainium2 Bass kernel for nn_ADFCell (adaptive decision-feedback equalizer).

Strategy: pure data-parallel over batch (8 examples/core on 8 cores).
Each core runs the L=4096-step recurrence with a 16-partition layout
p = mode*8 + example; the per-step chain is fused DVE ops
(scalar_tensor_tensor with accumulate, stream_shuffle for the cross-mode
energy sum), with ACT doing the two square roots.
"""

import numpy as np

import concourse.bacc as bacc
import concourse.bass as bass
import concourse.mybir as mybir
from concourse.bass_utils import run_bass_kernel_spmd
from concourse.tile import TileContext

Alu = mybir.AluOpType
f32 = mybir.dt.float32

B, L, TAPS, NM = 64, 4096, 32, 2
NCORES = 8
BC = B // NCORES          # 8 examples per core
P = NM * BC               # 16 partitions, p = i*BC + b
K = TAPS * NM * 2         # 128 floats = [ur(64) | ui(64)], k-order (j, tap)
TC = 64                   # time steps per chunk
LR_W = 1.0 / 2**6
LR_F = 1.0 / 2**7
GMAX = 30.0
EPS = 1e-9

_BUILT = {}


def _build_nc(l_total: int, static: bool = False):
    nc = bacc.Bacc("TRN2", target_bir_lowering=False, debug=False)
    u3 = nc.declare_dram_parameter("u3", [BC, l_total, K], f32, isOutput=False)
    u2s = nc.declare_dram_parameter("u2s", [BC, l_total, K], f32, isOutput=False)
    xm = nc.declare_dram_parameter("xm", [P, l_total, 8], f32, isOutput=False)
    al = nc.declare_dram_parameter("al", [P, l_total, 2], f32, isOutput=False)
    w0x = nc.declare_dram_parameter("w0x", [P, K], f32, isOutput=False)
    f0x = nc.declare_dram_parameter("f0x", [P, 4], f32, isOutput=False)
    ko = nc.declare_dram_parameter("ko", [P, l_total, 2], f32, isOutput=True)

    inv_lf2 = 1.0 / (LR_F * LR_F)

    with TileContext(nc) as tc:
        with (
            tc.tile_pool(name="state", bufs=1) as state,
            tc.tile_pool(name="io", bufs=2) as io,
            tc.tile_pool(name="scr", bufs=4) as scr,
        ):
            w1 = state.tile([P, K], f32)      # [wr | -wi]
            fT = state.tile([P, 2], f32)      # [f_r, f_i]
            sgn = state.tile([P, 2], f32)     # [1, -1]
            sgnB = state.tile([P, 4], f32)    # [-1, -1, 1, 1]
            m2t = state.tile([32, 2], f32)    # [-|v|^2, -partner] (32p for shuffle)
            nc.sync.dma_start(out=w1[:], in_=w0x[:])
            nc.sync.dma_start(out=fT[:], in_=f0x[:, 0:2])
            nc.vector.memset(sgn[:, 0:1], 1.0)
            nc.vector.memset(sgn[:, 1:2], -1.0)
            nc.vector.memset(sgnB[:, 0:2], -1.0)
            nc.vector.memset(sgnB[:, 2:4], 1.0)
            nc.vector.memset(m2t[:, :], 0.0)

            shuf_mask = [(p + BC) % P for p in range(P)] + [0] * 16

            from contextlib import nullcontext
            loop_cm = (nullcontext(0) if static and l_total == TC
                       else tc.For_i(0, l_total, TC))
            with loop_cm as t0:
                U3c = io.tile([P, TC, K], f32)
                U2c = io.tile([P, TC, K], f32)
                XMc = io.tile([P, TC, 8], f32)
                ALc = io.tile([P, TC, 2], f32)
                Kc = io.tile([P, TC, 2], f32)
                Vc = io.tile([P, TC, 2], f32)
                Fc = io.tile([P, TC, 2], f32)
                nc.sync.dma_start(out=U3c[0:BC], in_=u3[:, bass.ds(t0, TC), :])
                nc.sync.dma_start(out=U3c[BC:P], in_=u3[:, bass.ds(t0, TC), :])
                nc.sync.dma_start(out=U2c[0:BC], in_=u2s[:, bass.ds(t0, TC), :])
                nc.sync.dma_start(out=U2c[BC:P], in_=u2s[:, bass.ds(t0, TC), :])
                nc.gpsimd.dma_start(out=XMc[:], in_=xm[:, bass.ds(t0, TC), :])
                nc.gpsimd.dma_start(out=ALc[:], in_=al[:, bass.ds(t0, TC), :])

                for t in range(TC):
                    U3_t = U3c[:, t, :]
                    U2_t = U2c[:, t, :]
                    fprev = fT[:] if t == 0 else Fc[:, t - 1, 0:2]
                    gU = scr.tile([P, K], f32, tag="gU")
                    # w1 = [wr | wi]; U3 = [ur | -ui]; U2s = [ui | ur]
                    # v_r = sum(w1 o U3) = wr.ur - wi.ui
                    nc.vector.scalar_tensor_tensor(
                        out=gU[:], in0=w1[:], scalar=1.0, in1=U3_t,
                        op0=Alu.mult, op1=Alu.mult, accum_out=Vc[:, t, 0:1])
                    # v_i = sum(w1 o U2s) = wr.ui + wi.ur
                    gU2 = scr.tile([P, K], f32, tag="gU2")
                    nc.vector.scalar_tensor_tensor(
                        out=gU2[:], in0=w1[:], scalar=1.0, in1=U2_t,
                        op0=Alu.mult, op1=Alu.mult, accum_out=Vc[:, t, 1:2])

                    # |f|^2 -> |f| -> 1/|f|
                    sA = scr.tile([P, 4], f32, tag="sA")
                    g2 = scr.tile([P, 2], f32, tag="g2")
                    nc.scalar.activation(
                        out=g2[:], in_=fprev,
                        func=mybir.ActivationFunctionType.Square,
                        accum_out=sA[:, 0:1])
                    nc.scalar.sqrt(out=sA[:, 1:2], in_=sA[:, 0:1])
                    nc.vector.reciprocal_approx_fast(out=sA[:, 2:3], in_=sA[:, 1:2])

                    # psi4 = [psi_r, psi_i, psi_r, psi_i], psi = conj(f)/|f|
                    p4 = scr.tile([P, 4], f32, tag="p4")
                    f4 = fprev.broadcast_to((P, 2, 2)).rearrange("p a b -> p b a")
                    s4 = sgn[:].broadcast_to((P, 2, 2)).rearrange("p a b -> p b a")
                    nc.vector.scalar_tensor_tensor(
                        out=p4[:].rearrange("p (a b) -> p a b", a=2),
                        in0=f4, scalar=sA[:, 2:3], in1=s4,
                        op0=Alu.mult, op1=Alu.mult)

                    # ewn = v - x*psi ; cc = [c_r, c_i, -c_r, -c_i]
                    q4a = scr.tile([P, 4], f32, tag="q4a")
                    nc.vector.tensor_tensor(
                        out=q4a[:], in0=XMc[:, t, 0:4], in1=p4[:], op=Alu.mult)
                    xps = scr.tile([P, 2], f32, tag="xps")
                    nc.vector.scalar_tensor_tensor(
                        out=xps[:], in0=q4a[:, 0:4:2], scalar=0.0,
                        in1=q4a[:, 1:4:2], op0=Alu.add, op1=Alu.add)
                    ewn = scr.tile([P, 2], f32, tag="ewn")
                    nc.vector.tensor_tensor(
                        out=ewn[:], in0=Vc[:, t, 0:2], in1=xps[:], op=Alu.subtract)
                    cc = scr.tile([P, 4], f32, tag="cc")
                    ewn4 = ewn[:].broadcast_to((P, 2, 2)).rearrange(
                        "p a b -> p b a")
                    sgn4 = sgnB[:].rearrange("p (a b) -> p a b", a=2)
                    nc.vector.scalar_tensor_tensor(
                        out=cc[:].rearrange("p (a b) -> p a b", a=2),
                        in0=ewn4, scalar=ALc[:, t, 0:1], in1=sgn4,
                        op0=Alu.mult, op1=Alu.mult)

                    # w updates: w1 += c_r*U3 + c_i*U2s
                    # left: c_r*ur + c_i*ui = d(wr); right: -c_r*ui + c_i*ur = d(wi)
                    nc.vector.scalar_tensor_tensor(
                        out=w1[:], in0=U3_t, scalar=cc[:, 0:1],
                        in1=w1[:], op0=Alu.mult, op1=Alu.add)
                    nc.vector.scalar_tensor_tensor(
                        out=w1[:], in0=U2_t, scalar=cc[:, 1:2],
                        in1=w1[:], op0=Alu.mult, op1=Alu.add)

                    # energies
                    sB = scr.tile([P, 10], f32, tag="sB")
                    g2d = scr.tile([P, 2], f32, tag="g2d")
                    nc.vector.scalar_tensor_tensor(
                        out=g2d[:], in0=Vc[:, t, 0:2], scalar=-1.0,
                        in1=Vc[:, t, 0:2], op0=Alu.mult, op1=Alu.mult,
                        accum_out=m2t[0:P, 0:1])
                    nc.vector.stream_shuffle(
                        out=m2t[0:32, 1:2], in_=m2t[0:32, 0:1], mask=shuf_mask)
                    nc.vector.scalar_tensor_tensor(
                        out=sB[0:P, 2:3], in0=m2t[0:P, 1:2], scalar=-EPS,
                        in1=m2t[0:P, 0:1], op0=Alu.add, op1=Alu.add)

                    # n = x*conj(v) - f*|v_mode|^2
                    q4b = scr.tile([P, 4], f32, tag="q4b")
                    v4 = Vc[:, t, 0:2].broadcast_to((P, 2, 2)).rearrange(
                        "p a b -> p b a")
                    nc.vector.tensor_tensor(
                        out=q4b[:], in0=XMc[:, t, 4:8], in1=v4, op=Alu.mult)
                    xcv = scr.tile([P, 2], f32, tag="xcv")
                    nc.vector.scalar_tensor_tensor(
                        out=xcv[:], in0=q4b[:, 0:4:2], scalar=0.0,
                        in1=q4b[:, 1:4:2], op0=Alu.add, op1=Alu.add)
                    nn = scr.tile([P, 2], f32, tag="nn")
                    nc.vector.scalar_tensor_tensor(
                        out=nn[:], in0=fprev, scalar=m2t[0:P, 0:1], in1=xcv[:],
                        op0=Alu.mult, op1=Alu.add)

                    # ivq = LR_F / max(ve, |n|/30)
                    nc.vector.tensor_scalar(
                        out=sB[0:P, 4:5], in0=sB[0:P, 2:3], scalar1=sB[0:P, 2:3],
                        scalar2=inv_lf2, op0=Alu.mult, op1=Alu.mult)
                    g2e = scr.tile([P, 2], f32, tag="g2e")
                    nc.scalar.activation(
                        out=g2e[:], in_=nn[:],
                        func=mybir.ActivationFunctionType.Square,
                        accum_out=sB[0:P, 5:6])
                    nc.vector.tensor_scalar(
                        out=sB[0:P, 6:7], in0=sB[0:P, 5:6],
                        scalar1=inv_lf2 / (GMAX * GMAX),
                        scalar2=sB[0:P, 4:5], op0=Alu.mult, op1=Alu.max)
                    nc.scalar.sqrt(out=sB[0:P, 7:8], in_=sB[0:P, 6:7])
                    nc.vector.reciprocal_approx_fast(
                        out=sB[0:P, 8:9], in_=sB[0:P, 7:8])

                    # f += ivq * n  (written into the trajectory column)
                    nc.vector.scalar_tensor_tensor(
                        out=Fc[:, t, 0:2], in0=nn[:], scalar=sB[0:P, 8:9],
                        in1=fprev, op0=Alu.mult, op1=Alu.add)

                # bulk k = v*f over the whole chunk
                # f used at step t is Fc[t-1] (pre-update); build shifted view:
                # k(0) uses fT (chunk-start state), k(t) uses Fc[t-1]
                t1 = io.tile([P, TC, 2], f32)
                t2 = io.tile([P, TC, 2], f32)
                nc.vector.scalar_tensor_tensor(
                    out=t1[:, 0, 0:2], in0=fT[:], scalar=1.0,
                    in1=Vc[:, 0, 0:2], op0=Alu.mult, op1=Alu.mult)
                nc.vector.scalar_tensor_tensor(
                    out=t2[:, 0, 0:2], in0=fT[:], scalar=1.0,
                    in1=Vc[:, 0, 1::-1], op0=Alu.mult, op1=Alu.mult)
                nc.vector.tensor_tensor(
                    out=t1[:, 1:TC, :], in0=Fc[:, 0:TC - 1, :],
                    in1=Vc[:, 1:TC, :], op=Alu.mult)
                nc.vector.tensor_tensor(
                    out=t2[:, 1:TC, :], in0=Fc[:, 0:TC - 1, :],
                    in1=Vc[:, 1:TC, 1::-1], op=Alu.mult)
                nc.vector.tensor_tensor(
                    out=Kc[:, :, 0], in0=t1[:, :, 0], in1=t1[:, :, 1],
                    op=Alu.subtract)
                nc.vector.tensor_tensor(
                    out=Kc[:, :, 1], in0=t2[:, :, 0], in1=t2[:, :, 1],
                    op=Alu.add)
                # carry f state to next chunk
                nc.vector.tensor_copy(out=fT[:], in_=Fc[:, TC - 1, 0:2])
                nc.gpsimd.dma_start(out=ko[:, bass.ds(t0, TC), :], in_=Kc[:])

    nc.compile()
    return nc


def _host_prep(u_r, u_i, x_r, x_i, w0_r, w0_i, f0_r, f0_i):
    l_total = u_r.shape[1]
    # k-order (j, tap): [B, L, TAPS, NM] -> [B, L, NM, TAPS] -> [B, L, 64]
    urk = np.ascontiguousarray(u_r.transpose(0, 1, 3, 2)).reshape(B, l_total, 64)
    uik = np.ascontiguousarray(u_i.transpose(0, 1, 3, 2)).reshape(B, l_total, 64)
    u3h = np.concatenate([urk, -uik], axis=2)         # [B, L, 128]
    u2sh = np.concatenate([uik, urk], axis=2)         # [B, L, 128]
    ue = (u_r * u_r + u_i * u_i).sum(axis=(2, 3)) + EPS
    alpha = (LR_W / ue).astype(np.float32)            # [B, L]

    in_maps = []
    for c in range(NCORES):
        bs = slice(c * BC, (c + 1) * BC)
        xr = x_r[bs]; xi = x_i[bs]                    # [BC, L, NM]
        xmh = np.empty((P, l_total, 8), np.float32)
        alh = np.empty((P, l_total, 2), np.float32)
        w0h = np.empty((P, K), np.float32)
        f0h = np.empty((P, 4), np.float32)
        for i in range(NM):
            for b in range(BC):
                p = i * BC + b
                xmh[p, :, 0] = xr[b, :, i]
                xmh[p, :, 1] = -xi[b, :, i]
                xmh[p, :, 2] = xi[b, :, i]
                xmh[p, :, 3] = xr[b, :, i]
                xmh[p, :, 4] = xr[b, :, i]
                xmh[p, :, 5] = xi[b, :, i]
                xmh[p, :, 6] = xi[b, :, i]
                xmh[p, :, 7] = -xr[b, :, i]
                alh[p, :, 0] = alpha[c * BC + b]
                alh[p, :, 1] = -alpha[c * BC + b]
                w0h[p, 0:64] = w0_r[c * BC + b, i].reshape(64)
                w0h[p, 64:128] = w0_i[c * BC + b, i].reshape(64)
                f0h[p, 0] = f0_r[c * BC + b, i]
                f0h[p, 1] = f0_i[c * BC + b, i]
                f0h[p, 2] = f0_i[c * BC + b, i]
                f0h[p, 3] = f0_r[c * BC + b, i]
        in_maps.append({
            "u3": np.ascontiguousarray(u3h[bs]),
            "u2s": np.ascontiguousarray(u2sh[bs]),
            "xm": xmh, "al": alh, "w0x": w0h, "f0x": f0h,
        })
    return in_maps


def kernel(u_r, u_i, x_r, x_i, w0_r, w0_i, f0_r, f0_i, _want_results=False,
           _trace=False):
    l_total = u_r.shape[1]
    key = l_total
    if key not in _BUILT:
        _BUILT[key] = _build_nc(l_total)
    nc = _BUILT[key]
    in_maps = _host_prep(u_r, u_i, x_r, x_i, w0_r, w0_i, f0_r, f0_i)
    kw = {"trace": True} if _trace else {}
    res = run_bass_kernel_spmd(nc, in_maps, core_ids=list(range(NCORES)), **kw)
    out = np.empty((B, l_total, NM, 2), np.float32)
    for c in range(NCORES):
        ko = res.results[c]["ko"]                      # [P, L, 2]
        for i in range(NM):
            out[c * BC:(c + 1) * BC, :, i, 0] = ko[i * BC:(i + 1) * BC, :, 0]
            out[c * BC:(c + 1) * BC, :, i, 1] = ko[i * BC:(i + 1) * BC, :, 1]
    if _want_results:
        return out, res
    return out

